# revision 11
# baseline (speedup 1.0000x reference)
"""MoD-router FFN kernel for 8 TRN2 NeuronCores (self-contained).

Math note: the reference applies softmax over a size-1 axis, which yields
all-ones scores for ANY input; jax.lax.top_k is stable, so the selected
token indices are always [0..NUM_TOKENS) per batch row. The router weights
(Wp, bp) therefore cannot affect the output, and the kernel computes

    out = gelu_tanh(x[:, :2048, :] @ W1 + b1) @ W2 + b2

Sharding: data-parallel over the 4*2048 = 8192 selected token rows ->
1024 rows per core. Each core runs a fused transposed FFN:
  H^T = gelu(W1^T @ X^T + b1)   (per F-block of 512, kept in SBUF)
  out^T += W2_blk^T @ H^T_blk   (accumulated in SBUF fp32)
Matmuls run in float32r (full PE rate at N=512, ~1.5e-4 rel err).
"""

import numpy as np

B, S, D, F = 4, 4096, 2048, 8192
NUM_TOKENS = 2048
NCORES = 8
ROWS = (B * NUM_TOKENS) // NCORES     # 1024 rows per core
P = 128
KT_D = D // P                         # 16 k-subtiles over D
FT = F // P                           # 64 f-tiles
FB = 16                               # F-blocks of 512
FSUB = 4                              # f-subtiles per block
DT = D // P                           # 16 d-tiles
NCH = ROWS // 512                     # 2 row chunks of 512
KS_W2 = 4                             # k-subtiles per F-block in FFN2

_CACHE = {}


def _build():
    import concourse.bass as bass
    import concourse.mybir as mybir
    import concourse.tile as tile
    from concourse import bacc

    f32 = mybir.dt.float32
    f32r = mybir.dt.float32r

    nc = bacc.Bacc()
    xt = nc.declare_dram_parameter("xt", [KT_D, P, ROWS], f32r, isOutput=False)
    w1 = nc.declare_dram_parameter("w1", [FT, P, KT_D, P], f32r, isOutput=False)
    w2 = nc.declare_dram_parameter("w2", [FB, DT, P, KS_W2, P], f32r, isOutput=False)
    b1 = nc.declare_dram_parameter("b1", [P, FT], f32, isOutput=False)
    b2 = nc.declare_dram_parameter("b2", [P, DT], f32, isOutput=False)
    out = nc.declare_dram_parameter("out", [DT, P, ROWS], f32, isOutput=True)

    with tile.TileContext(nc) as tc:
        with (
            tc.tile_pool(name="resident", bufs=1) as res_pool,
            tc.tile_pool(name="w1p", bufs=4) as w1p,
            tc.tile_pool(name="w2p", bufs=4) as w2p,
            tc.tile_pool(name="htp", bufs=8) as htp,
            tc.tile_pool(name="ps1", bufs=4, space="PSUM") as ps1,
            tc.tile_pool(name="ps2", bufs=4, space="PSUM") as ps2,
        ):
            # resident tiles; DMA issue order matters for startup: the first
            # F-block's weights go first, then XT streams in k order so the
            # k-outer warmup block below can compute behind the DMA wave.
            xt_sb = [res_pool.tile([P, ROWS], f32r, name=f"xt{k}") for k in range(KT_D)]
            w1_warm = [w1p.tile([P, KT_D * P], f32r, name=f"w1t_{ft}", tag="w1t")
                       for ft in range(FSUB)]
            b1_sb = res_pool.tile([P, FT], f32, name="b1sb")
            b2_sb = res_pool.tile([P, DT], f32, name="b2sb")
            # Startup DMA staging. A single DMA queue moves ~60 GB/s, so big
            # early tiles are split across queues, and XT streams in gated
            # sub-waves ordered by first use (so early tiles get full BW
            # instead of an equal share with tiles needed much later).
            def load_w1(dst, ft, nsplit=1):
                kstep = KT_D // nsplit
                for i in range(nsplit):
                    nc.sync.dma_start(
                        out=dst[:, i * kstep * P:(i + 1) * kstep * P],
                        in_=w1[ft, :, i * kstep:(i + 1) * kstep, :].rearrange("p k c -> p (k c)"))

            def load_xt(k, nsplit=1):
                half = ROWS // nsplit
                for i in range(nsplit):
                    nc.sync.dma_start(out=xt_sb[k][:, i * half:(i + 1) * half],
                                      in_=xt[k, :, i * half:(i + 1) * half])

            # wave 1: first chains' operands, maximally queue-parallel
            nc.sync.dma_start(out=b1_sb[:], in_=b1[:])
            load_w1(w1_warm[0], 0, nsplit=4)
            load_w1(w1_warm[1], 1, nsplit=4)
            load_xt(0, nsplit=2)
            load_xt(1, nsplit=2)

            # gate helper: 1-element DVE copy reading a prior-wave tile and
            # writing the next wave's destination; the next DMA WAW-waits.
            def gated(dst_tile, src_tile):
                nc.vector.tensor_copy(dst_tile[:1, :1], src_tile[:1, :1])

            waves = [[2, 3, 4, 5], [6, 7, 8, 9], [10, 11, 12, 13], [14, 15]]
            prev_last = xt_sb[1]
            for wave in waves:
                src = prev_last
                for k in wave:
                    gated(xt_sb[k], src)
                    load_xt(k, nsplit=2)
                prev_last = xt_sb[wave[-1]]
            gated(w1_warm[2], xt_sb[9])
            load_w1(w1_warm[2], 2, nsplit=2)
            gated(w1_warm[3], xt_sb[13])
            load_w1(w1_warm[3], 3, nsplit=2)
            gated(b2_sb, b1_sb)
            nc.sync.dma_start(out=b2_sb[:], in_=b2[:])

            # out accumulator, initialized to broadcast b2 (scale=0 trick)
            oacc = [res_pool.tile([P, ROWS], f32, name=f"oacc{d}") for d in range(DT)]
            for d in range(DT):
                nc.scalar.activation(
                    oacc[d][:], xt_sb[0][:].bitcast(f32),
                    mybir.ActivationFunctionType.Identity,
                    bias=b2_sb[:, d:d + 1], scale=0.0,
                )

            for fb in range(FB):
                ht = []
                if fb == 0:
                    # warmup block: k-outer over 4 concurrent psum chains
                    # (2 f-subtiles x 2 row chunks per pass) so matmuls start
                    # as soon as xt_sb[k] lands instead of waiting for all XT.
                    for fs in range(FSUB):
                        ht.append(htp.tile([P, ROWS], f32r, name=f"ht_{fs}", tag="ht"))
                    for half in range(2):
                        chains = [(half * 2 + i, n) for i in range(2) for n in range(NCH)]
                        psums = {
                            c: ps1.tile([P, 512], f32, name=f"ps1w_{c[0]}_{c[1]}", tag="ps1")
                            for c in chains
                        }
                        for k in range(KT_D):
                            for fs, n in chains:
                                nc.tensor.matmul(
                                    psums[(fs, n)][:],
                                    w1_warm[fs][:, k * P:(k + 1) * P],
                                    xt_sb[k][:, n * 512:(n + 1) * 512],
                                    start=(k == 0), stop=(k == KT_D - 1),
                                )
                        for fs, n in chains:
                            nc.scalar.activation(
                                ht[fs][:, n * 512:(n + 1) * 512], psums[(fs, n)][:],
                                mybir.ActivationFunctionType.Gelu_apprx_tanh,
                                bias=b1_sb[:, fs:fs + 1],
                            )
                else:
                    for fs in range(FSUB):
                        ft = fb * FSUB + fs
                        w1_sb = w1p.tile([P, KT_D * P], f32r, name=f"w1t_{ft}", tag="w1t")
                        nc.sync.dma_start(out=w1_sb[:], in_=w1[ft].rearrange("p k c -> p (k c)"))
                        ht_t = htp.tile([P, ROWS], f32r, name=f"ht_{ft}", tag="ht")
                        for n in range(NCH):
                            psum = ps1.tile([P, 512], f32, name=f"ps1_{ft}_{n}", tag="ps1")
                            for k in range(KT_D):
                                nc.tensor.matmul(
                                    psum[:],
                                    w1_sb[:, k * P:(k + 1) * P],
                                    xt_sb[k][:, n * 512:(n + 1) * 512],
                                    start=(k == 0), stop=(k == KT_D - 1),
                                )
                            nc.scalar.activation(
                                ht_t[:, n * 512:(n + 1) * 512], psum[:],
                                mybir.ActivationFunctionType.Gelu_apprx_tanh,
                                bias=b1_sb[:, ft:ft + 1],
                            )
                        ht.append(ht_t)

                for d in range(DT):
                    w2_sb = w2p.tile([P, KS_W2 * P], f32r, name=f"w2t_{fb}_{d}", tag="w2t")
                    nc.sync.dma_start(out=w2_sb[:], in_=w2[fb, d].rearrange("p k c -> p (k c)"))
                    for n in range(NCH):
                        psum2 = ps2.tile([P, 512], f32, name=f"ps2_{fb}_{d}_{n}", tag="ps2")
                        for ks in range(KS_W2):
                            nc.tensor.matmul(
                                psum2[:],
                                w2_sb[:, ks * P:(ks + 1) * P],
                                ht[ks][:, n * 512:(n + 1) * 512],
                                start=(ks == 0), stop=(ks == KS_W2 - 1),
                            )
                        nc.vector.tensor_add(
                            oacc[d][:, n * 512:(n + 1) * 512],
                            oacc[d][:, n * 512:(n + 1) * 512],
                            psum2[:],
                        )

            for d in range(DT):
                nc.sync.dma_start(out=out[d], in_=oacc[d][:])

    nc.compile()
    return nc


def _get_nc():
    if "nc" not in _CACHE:
        _CACHE["nc"] = _build()
    return _CACHE["nc"]


def kernel(x, Wp, bp, W1, b1, W2, b2, **_unused):
    from concourse.bass_utils import run_bass_kernel_spmd

    x = np.asarray(x, dtype=np.float32)
    W1 = np.asarray(W1, dtype=np.float32)
    W2 = np.asarray(W2, dtype=np.float32)
    b1 = np.asarray(b1, dtype=np.float32)
    b2 = np.asarray(b2, dtype=np.float32)

    # host-side shard + layout prep
    xs = x[:, :NUM_TOKENS, :].reshape(B * NUM_TOKENS, D)         # [8192, 2048]
    w1h = np.ascontiguousarray(
        W1.reshape(KT_D, P, FT, P).transpose(2, 1, 0, 3))        # [ft, p, k, c]
    w2h = np.ascontiguousarray(
        W2.reshape(FB, KS_W2, P, DT, P).transpose(0, 3, 2, 1, 4))  # [fb, d, p, ks, c]
    b1h = np.ascontiguousarray(b1.reshape(FT, P).T)              # [p, ft]
    b2h = np.ascontiguousarray(b2.reshape(DT, P).T)              # [p, d]

    in_maps = []
    for c in range(NCORES):
        xc = xs[c * ROWS:(c + 1) * ROWS]                         # [1024, 2048]
        xth = np.ascontiguousarray(xc.T.reshape(KT_D, P, ROWS))  # [k, p, n]
        in_maps.append({"xt": xth, "w1": w1h, "w2": w2h, "b1": b1h, "b2": b2h})

    nc = _get_nc()
    res = run_bass_kernel_spmd(nc, in_maps, list(range(NCORES)))

    out = np.empty((B * NUM_TOKENS, D), dtype=np.float32)
    for c in range(NCORES):
        oc = res.results[c]["out"]                               # [d, p, n]
        out[c * ROWS:(c + 1) * ROWS] = oc.reshape(D, ROWS).T
    return out.reshape(B, NUM_TOKENS, D)
